# revision 1
# baseline (speedup 1.0000x reference)
"""Trainium2 Bass kernel for nn_CustomConv2D (degenerate conv: only the last
input channel contributes; 3x3 VALID conv -> 64 out channels + bias).

Strategy:
  - Host: slice x_padded[:, -1] (the only channel the reference uses), build
    the 9-row im2col matrix per batch (cheap: 29 MB total), shard batch dim
    across 8 cores (8 batches per core).
  - Device (per core): one [128, 3136] moving tile per batch PAIR holds the
    pair's im2col matrix [18, 12544] split into 4 pixel segments placed at
    partition offsets 0/32/64/96 (one contiguous DMA, full port spread).
    Stationary weight [128, 128] is block-diagonal over the pair (cols 0-63
    batch A channels, 64-127 batch B) and replicated at the 4 partition
    offsets. Each segment runs 7 fp32 matmuls (N=448) at tile_position
    (32s, 0) -> PSUM [128, 448]; bias is fused into the PSUM->SBUF
    evacuation (alternating VectorE tensor_scalar_add / ScalarE activation
    Identity), and each segment's [128, 3136] staging tile streams out as a
    1.6 MiB DMA.
"""

import sys

if "/opt/trn_rl_repo" not in sys.path:
    sys.path.insert(0, "/opt/trn_rl_repo")

import numpy as np

B, CIN, COUT, KS = 64, 64, 64, 3
H, W, HP, WP = 112, 112, 114, 114
NPIX = H * W          # 12544
IMG = HP * WP         # 12996
NCORES = 8
BL = B // NCORES      # 8 local batches per core
PAIRS = BL // 2       # 4
KDIM = 2 * KS * KS    # 18
NSEG = 4              # pixel segments per pair (partition offsets 0/32/64/96)
SEGW = NPIX // NSEG   # 3136
NT = 448              # pixels per matmul; 7 * 448 == 3136, fits one PSUM bank
TPS = SEGW // NT      # 7 matmul tiles per segment

_CACHE = {}


def _build_bass():
    import concourse.bass as bass
    import concourse.bacc as bacc
    import concourse.mybir as mybir
    from concourse.tile import TileContext

    f32 = mybir.dt.float32
    f32r = mybir.dt.float32r
    # Bacc (not plain Bass): its compile() runs move_matmul_waits_to_ldweights
    # + generate_event_semaphores, without which walrus rejects any sync wait
    # on a Matmult ("Too many sync wait commands").
    nc = bacc.Bacc("TRN2", target_bir_lowering=False, debug=False)
    mv = nc.declare_dram_parameter("mv", [PAIRS, 128, SEGW], f32r,
                                   isOutput=False)
    w2 = nc.declare_dram_parameter("w2", [128, 128], f32r, isOutput=False)
    b2 = nc.declare_dram_parameter("b2", [128, 1], f32, isOutput=False)
    out = nc.declare_dram_parameter("out", [BL * COUT, NPIX], f32,
                                    isOutput=True)

    with TileContext(nc) as tc:
        with (
            tc.tile_pool(name="consts", bufs=1) as consts,
            tc.tile_pool(name="movp", bufs=2) as movp,
            tc.tile_pool(name="stagep", bufs=10) as stagep,
            tc.tile_pool(name="psump", bufs=8, space="PSUM") as psump,
        ):
            w2_t = consts.tile([128, 128], f32r)
            nc.scalar.dma_start(out=w2_t[:], in_=w2[:])
            b2_t = consts.tile([128, 1], f32)
            nc.sync.dma_start(out=b2_t[:], in_=b2[:])




            tidx = 0
            for pair in range(PAIRS):
                # 32-row groups arrive fully (rows 18-31 zero-filled from
                # host; their weight rows are zero too). Per-seg DMAs let
                # each segment's matmuls start as soon as its rows land.
                mov = movp.tile([128, SEGW + 32], f32r, tag="mov")
                for s4 in range(NSEG):
                    nc.scalar.dma_start(
                        out=mov[32 * s4:32 * (s4 + 1), 0:SEGW],
                        in_=mv[pair, 32 * s4:32 * (s4 + 1), :])

                # t-major emission: consecutive matmuls hit different
                # 32-row groups, so up to 4 run concurrently in the PE array.
                stages = [stagep.tile([128, SEGW], f32, tag="stage",
                                      name=f"stage_{pair}_{s}")
                          for s in range(NSEG)]
                for t in range(TPS):
                    n0 = t * NT
                    for seg in range(NSEG):
                        p0 = 32 * seg
                        ps = psump.tile([128, NT], f32, tag="ps")
                        nc.tensor.matmul(ps[:, :],
                                         w2_t[p0:p0 + KDIM, :],
                                         mov[p0:p0 + KDIM, n0:n0 + NT],
                                         start=True, stop=True,
                                         tile_position=(p0, 0))
                        # PSUM -> SBUF with fused bias add; alternate engines.
                        if tidx % 2 == 0:
                            nc.vector.tensor_scalar_add(
                                stages[seg][:, n0:n0 + NT], ps[:, :],
                                b2_t[:, :])
                        else:
                            nc.scalar.activation(
                                stages[seg][:, n0:n0 + NT], ps[:, :],
                                mybir.ActivationFunctionType.Identity,
                                bias=b2_t[:, :])
                        tidx += 1
                    if t == 3:
                        # first 4 columns-of-448 of every stage are done:
                        # start draining while t=4..6 compute
                        for seg in range(NSEG):
                            nc.sync.dma_start(
                                out=out[pair * 128:(pair + 1) * 128,
                                        seg * SEGW:seg * SEGW + 4 * NT],
                                in_=stages[seg][:, 0:4 * NT])
                for seg in range(NSEG):
                    nc.sync.dma_start(
                        out=out[pair * 128:(pair + 1) * 128,
                                seg * SEGW + 4 * NT:(seg + 1) * SEGW],
                        in_=stages[seg][:, 4 * NT:SEGW])
    nc.compile()
    return nc


def _get_nc():
    if "nc" not in _CACHE:
        _CACHE["nc"] = _build_bass()
    return _CACHE["nc"]


def _prep_inputs(x_padded, weight, bias):
    x = np.asarray(x_padded, dtype=np.float32)
    wt = np.asarray(weight, dtype=np.float32)
    bs = np.asarray(bias, dtype=np.float32)

    xs3 = x[:, -1, :, :]                              # [64, 114, 114]
    win = np.lib.stride_tricks.sliding_window_view(xs3, (KS, KS), axis=(1, 2))
    # [64, 112, 112, 3, 3] -> [64, 9, 12544] with row k = (i, j) shift
    mov_all = win.transpose(0, 3, 4, 1, 2).reshape(B, KS * KS, NPIX)
    # [cores, pairs, 18, NSEG, SEGW] -> [cores, pairs, NSEG, 32, SEGW]
    mov_r = mov_all.reshape(NCORES, PAIRS, KDIM, NSEG, SEGW).transpose(0, 1, 3, 2, 4)
    mov_h = np.zeros((NCORES, PAIRS, NSEG, 32, SEGW), np.float32)
    mov_h[:, :, :, :KDIM, :] = mov_r
    mov_h = mov_h.reshape(NCORES, PAIRS, 128, SEGW)

    wl = np.ascontiguousarray(wt[:, -1, :, :]).reshape(COUT, KS * KS)
    w2 = np.zeros((128, 128), np.float32)
    for s in range(NSEG):
        w2[32 * s: 32 * s + 9, 0:64] = wl.T
        w2[32 * s + 9: 32 * s + 18, 64:128] = wl.T
    b2 = np.tile(bs, 2).reshape(128, 1).astype(np.float32)
    return mov_h, w2, b2


def kernel(x_padded, weight, bias, in_height=112, in_width=112, **_unused):
    from concourse.bass_utils import run_bass_kernel_spmd

    mov_h, w2, b2 = _prep_inputs(x_padded, weight, bias)
    nc = _get_nc()
    in_maps = [
        {"mv": mov_h[c], "w2": w2, "b2": b2}
        for c in range(NCORES)
    ]
    res = run_bass_kernel_spmd(nc, in_maps, core_ids=list(range(NCORES)))
    outs = [
        np.asarray(res.results[c]["out"]).reshape(BL, COUT, H, W)
        for c in range(NCORES)
    ]
    return np.concatenate(outs, axis=0)



# revision 2
# speedup vs baseline: 1.2078x; 1.2078x over previous
"""Trainium2 Bass kernel for nn_CustomConv2D (degenerate conv: only the last
input channel contributes; 3x3 VALID conv -> 64 out channels + bias).

Strategy (v2 — DMA-stream optimized):
  - Host: slice x_padded[:, -1], build the 9-row im2col matrix per batch,
    pack batch PAIRS into 18-row blocks (rows 0-8 img A, 9-17 img B), cast
    to bf16 (tolerance 2e-2 >> bf16 error ~4e-3). Only the 18 useful rows
    per 32-row PE quadrant are shipped: 1.81 MB/core vs 6.42 MB before.
  - Device (per core): prefetch ALL input up front (16 small DMAs issued
    from the otherwise-idle GpSimd engine), then stream output continuously:
    for each pair, 7x4 bf16 matmuls [18 -> 128, 448] at quadrant offsets
    0/32/64/96 -> PSUM f32; bias fused into the PSUM->SBUF evacuation
    (alternating VectorE tensor_scalar_add / ScalarE activation Identity);
    drains are fine-grained (896-col chunks issued right after their
    evictions complete, alternating Sync/GpSimd issue engines) so the
    25.7 MB/core output write never stalls on pair boundaries.
"""

import sys

if "/opt/trn_rl_repo" not in sys.path:
    sys.path.insert(0, "/opt/trn_rl_repo")

import numpy as np

B, CIN, COUT, KS = 64, 64, 64, 3
H, W, HP, WP = 112, 112, 114, 114
NPIX = H * W          # 12544
NCORES = 8
BL = B // NCORES      # 8 local batches per core
PAIRS = BL // 2       # 4
KDIM = 2 * KS * KS    # 18 (9 taps x 2 images, block-diagonal weights)
NSEG = 4              # pixel segments per pair (partition offsets 0/32/64/96)
SEGW = NPIX // NSEG   # 3136
NT = 448              # pixels per matmul; 7 * 448 == 3136, fits one PSUM bank
TPS = SEGW // NT      # 7 matmul tiles per segment

_CACHE = {}


def _build_bass():
    import concourse.bass as bass
    import concourse.bacc as bacc
    import concourse.mybir as mybir
    from concourse.tile import TileContext

    f32 = mybir.dt.float32
    bf16 = mybir.dt.bfloat16
    # Bacc (not plain Bass): its compile() runs move_matmul_waits_to_ldweights
    # + generate_event_semaphores, without which walrus rejects any sync wait
    # on a Matmult ("Too many sync wait commands").
    nc = bacc.Bacc("TRN2", target_bir_lowering=False, debug=False)
    mv = nc.declare_dram_parameter("mv", [PAIRS, NSEG, KDIM, SEGW], bf16,
                                   isOutput=False)
    w2 = nc.declare_dram_parameter("w2", [128, 128], bf16, isOutput=False)
    b2 = nc.declare_dram_parameter("b2", [128, 1], f32, isOutput=False)
    out = nc.declare_dram_parameter("out", [BL * COUT, NPIX], f32,
                                    isOutput=True)

    with TileContext(nc) as tc:
        with (
            tc.tile_pool(name="consts", bufs=1) as consts,
            tc.tile_pool(name="movp", bufs=PAIRS) as movp,
            tc.tile_pool(name="stagep", bufs=8) as stagep,
            tc.tile_pool(name="psump", bufs=8, space="PSUM") as psump,
        ):
            w2_t = consts.tile([128, 128], bf16)
            nc.scalar.dma_start(out=w2_t[:], in_=w2[:])
            b2_t = consts.tile([128, 1], f32)
            nc.scalar.dma_start(out=b2_t[:], in_=b2[:])

            # Prefetch every pair's im2col block. Only the 18 useful rows of
            # each 32-row quadrant are transferred; rows 18-31 are never read
            # by the matmuls. GpSimd has no other early work, so the ~16
            # issue slots don't delay the compute engines.
            movs = []
            for pair in range(PAIRS):
                mov = movp.tile([128, SEGW + 32], bf16, tag="mov",
                                name=f"mov{pair}")
                for s in range(NSEG):
                    nc.gpsimd.dma_start(
                        out=mov[32 * s:32 * s + KDIM, 0:SEGW],
                        in_=mv[pair, s])
                movs.append(mov)

            tidx = 0
            didx = 0
            for pair in range(PAIRS):
                stages = [stagep.tile([128, SEGW], f32, tag="stage",
                                      name=f"stage_{pair}_{s}")
                          for s in range(NSEG)]
                for t in range(TPS):
                    n0 = t * NT
                    # t-major emission: consecutive matmuls hit different
                    # 32-row quadrants, so up to 4 overlap in the PE array.
                    for seg in range(NSEG):
                        p0 = 32 * seg
                        ps = psump.tile([128, NT], f32, tag="ps")
                        nc.tensor.matmul(ps[:, :],
                                         w2_t[p0:p0 + KDIM, :],
                                         movs[pair][p0:p0 + KDIM,
                                                    n0:n0 + NT],
                                         start=True, stop=True,
                                         tile_position=(p0, 0))
                        # PSUM -> SBUF with fused bias add; alternate engines.
                        if tidx % 2 == 0:
                            nc.vector.tensor_scalar_add(
                                stages[seg][:, n0:n0 + NT], ps[:, :],
                                b2_t[:, :])
                        else:
                            nc.scalar.activation(
                                stages[seg][:, n0:n0 + NT], ps[:, :],
                                mybir.ActivationFunctionType.Identity,
                                bias=b2_t[:, :])
                        tidx += 1
                    # Fine-grained drains: push each 896-col chunk to HBM as
                    # soon as its two evictions land, so the output stream
                    # never waits for a whole pair to finish.
                    if t in (1, 3, 5):
                        c0 = (t - 1) * NT
                        for seg in range(NSEG):
                            eng = nc.sync if (pair == 0 or didx % 2 == 0) \
                                else nc.gpsimd
                            eng.dma_start(
                                out=out[pair * 128:(pair + 1) * 128,
                                        seg * SEGW + c0:
                                        seg * SEGW + c0 + 2 * NT],
                                in_=stages[seg][:, c0:c0 + 2 * NT])
                            didx += 1
                for seg in range(NSEG):
                    eng = nc.sync if didx % 2 == 0 else nc.gpsimd
                    eng.dma_start(
                        out=out[pair * 128:(pair + 1) * 128,
                                seg * SEGW + 6 * NT:(seg + 1) * SEGW],
                        in_=stages[seg][:, 6 * NT:SEGW])
                    didx += 1
    nc.compile()
    return nc


def _get_nc():
    if "nc" not in _CACHE:
        _CACHE["nc"] = _build_bass()
    return _CACHE["nc"]


def _prep_inputs(x_padded, weight, bias):
    import ml_dtypes

    x = np.asarray(x_padded, dtype=np.float32)
    wt = np.asarray(weight, dtype=np.float32)
    bs = np.asarray(bias, dtype=np.float32)

    xs3 = x[:, -1, :, :]                              # [64, 114, 114]
    win = np.lib.stride_tricks.sliding_window_view(xs3, (KS, KS), axis=(1, 2))
    # [64, 112, 112, 3, 3] -> [64, 9, 12544] with row k = (i, j) shift
    mov_all = win.transpose(0, 3, 4, 1, 2).reshape(B, KS * KS, NPIX)
    # [cores, pairs, img2, 9, seg, SEGW] -> [cores, pairs, seg, (img2, 9), SEGW]
    mov_r = mov_all.reshape(NCORES, PAIRS, 2, KS * KS, NSEG, SEGW)
    mov_h = np.ascontiguousarray(
        mov_r.transpose(0, 1, 4, 2, 3, 5)
    ).reshape(NCORES, PAIRS, NSEG, KDIM, SEGW).astype(ml_dtypes.bfloat16)

    wl = np.ascontiguousarray(wt[:, -1, :, :]).reshape(COUT, KS * KS)
    w2 = np.zeros((128, 128), np.float32)
    for s in range(NSEG):
        w2[32 * s: 32 * s + 9, 0:64] = wl.T
        w2[32 * s + 9: 32 * s + 18, 64:128] = wl.T
    w2 = w2.astype(ml_dtypes.bfloat16)
    b2 = np.tile(bs, 2).reshape(128, 1).astype(np.float32)
    return mov_h, w2, b2


def kernel(x_padded, weight, bias, in_height=112, in_width=112, **_unused):
    from concourse.bass_utils import run_bass_kernel_spmd

    mov_h, w2, b2 = _prep_inputs(x_padded, weight, bias)
    nc = _get_nc()
    in_maps = [
        {"mv": mov_h[c], "w2": w2, "b2": b2}
        for c in range(NCORES)
    ]
    res = run_bass_kernel_spmd(nc, in_maps, core_ids=list(range(NCORES)))
    outs = [
        np.asarray(res.results[c]["out"]).reshape(BL, COUT, H, W)
        for c in range(NCORES)
    ]
    return np.concatenate(outs, axis=0)


# revision 4
# speedup vs baseline: 1.2119x; 1.0034x over previous
"""Trainium2 Bass kernel for nn_CustomConv2D (degenerate conv: only the last
input channel contributes; 3x3 VALID conv -> 64 out channels + bias).

Strategy (v3 — smooth-streamed DMA):
  - Host: slice x_padded[:, -1], build the 9-row im2col matrix per batch,
    pack batch PAIRS into 18-row blocks (rows 0-8 img A, 9-17 img B), cast
    to bf16 (tolerance 2e-2 >> bf16 error ~7e-4). Only the 18 useful rows
    per 32-row PE quadrant are shipped: 1.81 MB/core.
  - Device (per core): all input prefetched immediately, with the issue
    cost (~0.6us per dma_start) spread over four engines so pair 0 lands
    within ~2us. Compute is seg-major (7 matmuls per 32-row quadrant, then
    the next quadrant) so each seg's staging tile completes early and its
    drain can start 1/4 of the way through a pair instead of all four
    bunching at the pair boundary. Each seg drains in two halves (ready
    after t=3 and t=6 evictions), ~0.8 MB per drain, issued alternately
    from Sync and GpSimd; 12 staging buffers keep 3 pairs in flight so the
    25.7 MB/core output write streams at the ~400 GB/s HBM roofline without
    pipeline bubbles. Bias is fused into the PSUM->SBUF evacuation
    (alternating VectorE tensor_scalar_add / ScalarE activation Identity).
"""

import sys

if "/opt/trn_rl_repo" not in sys.path:
    sys.path.insert(0, "/opt/trn_rl_repo")

import numpy as np

B, CIN, COUT, KS = 64, 64, 64, 3
H, W, HP, WP = 112, 112, 114, 114
NPIX = H * W          # 12544
NCORES = 8
BL = B // NCORES      # 8 local batches per core
PAIRS = BL // 2       # 4
KDIM = 2 * KS * KS    # 18 (9 taps x 2 images, block-diagonal weights)
NSEG = 4              # pixel segments per pair (partition offsets 0/32/64/96)
SEGW = NPIX // NSEG   # 3136
NT = 448              # pixels per matmul; 7 * 448 == 3136, fits one PSUM bank
TPS = SEGW // NT      # 7 matmul tiles per segment
HALF = 4 * NT         # first drain half: cols [0, 1792), second [1792, 3136)

_CACHE = {}


def _build_bass():
    import concourse.bass as bass
    import concourse.bacc as bacc
    import concourse.mybir as mybir
    from concourse.tile import TileContext

    f32 = mybir.dt.float32
    bf16 = mybir.dt.bfloat16
    # Bacc (not plain Bass): its compile() runs move_matmul_waits_to_ldweights
    # + generate_event_semaphores, without which walrus rejects any sync wait
    # on a Matmult ("Too many sync wait commands").
    nc = bacc.Bacc("TRN2", target_bir_lowering=False, debug=False)
    mv = nc.declare_dram_parameter("mv", [PAIRS, NSEG, KDIM, SEGW], bf16,
                                   isOutput=False)
    w2 = nc.declare_dram_parameter("w2", [128, 128], bf16, isOutput=False)
    b2 = nc.declare_dram_parameter("b2", [128, 1], f32, isOutput=False)
    out = nc.declare_dram_parameter("out", [BL * COUT, NPIX], f32,
                                    isOutput=True)

    with TileContext(nc) as tc:
        with (
            tc.tile_pool(name="consts", bufs=1) as consts,
            tc.tile_pool(name="movp", bufs=PAIRS) as movp,
            tc.tile_pool(name="stagep", bufs=12) as stagep,
            tc.tile_pool(name="psump", bufs=8, space="PSUM") as psump,
        ):
            w2_t = consts.tile([128, 128], bf16)
            nc.scalar.dma_start(out=w2_t[:], in_=w2[:])
            b2_t = consts.tile([128, 1], f32)
            nc.scalar.dma_start(out=b2_t[:], in_=b2[:])

            # Prefetch every pair's im2col block (18 useful rows per 32-row
            # quadrant). Only SP/Activation/GpSimd can issue DMAs; issue cost
            # is ~0.6us per dma_start, so pair 0's segments are spread across
            # all three while the rest (needed 16us+ later) queue on GpSimd.
            in_engines = {(0, 0): nc.scalar, (0, 1): nc.sync,
                          (0, 2): nc.gpsimd, (0, 3): nc.gpsimd}
            movs = []
            for pair in range(PAIRS):
                mov = movp.tile([128, SEGW + 32], bf16, tag="mov",
                                name=f"mov{pair}")
                for s in range(NSEG):
                    eng = in_engines.get((pair, s), nc.gpsimd)
                    eng.dma_start(out=mov[32 * s:32 * s + KDIM, 0:SEGW],
                                  in_=mv[pair, s])
                movs.append(mov)

            tidx = 0
            didx = 0
            for pair in range(PAIRS):
                stages = [stagep.tile([128, SEGW], f32, tag="stage",
                                      name=f"stage_{pair}_{s}")
                          for s in range(NSEG)]
                for seg in range(NSEG):
                    p0 = 32 * seg
                    for t in range(TPS):
                        n0 = t * NT
                        ps = psump.tile([128, NT], f32, tag="ps")
                        nc.tensor.matmul(ps[:, :],
                                         w2_t[p0:p0 + KDIM, :],
                                         movs[pair][p0:p0 + KDIM,
                                                    n0:n0 + NT],
                                         start=True, stop=True,
                                         tile_position=(p0, 0))
                        # PSUM -> SBUF with fused bias add; alternate engines.
                        if tidx % 2 == 0:
                            nc.vector.tensor_scalar_add(
                                stages[seg][:, n0:n0 + NT], ps[:, :],
                                b2_t[:, :])
                        else:
                            nc.scalar.activation(
                                stages[seg][:, n0:n0 + NT], ps[:, :],
                                mybir.ActivationFunctionType.Identity,
                                bias=b2_t[:, :])
                        tidx += 1
                        # Drain each half as soon as its evictions land so
                        # the output stream stays continuous.
                        if t == 3 or t == TPS - 1:
                            c0 = 0 if t == 3 else HALF
                            c1 = HALF if t == 3 else SEGW
                            eng = nc.sync if didx % 2 == 0 else nc.gpsimd
                            eng.dma_start(
                                out=out[pair * 128:(pair + 1) * 128,
                                        seg * SEGW + c0:seg * SEGW + c1],
                                in_=stages[seg][:, c0:c1])
                            didx += 1
    nc.compile()
    return nc


def _get_nc():
    if "nc" not in _CACHE:
        _CACHE["nc"] = _build_bass()
    return _CACHE["nc"]


def _prep_inputs(x_padded, weight, bias):
    import ml_dtypes

    x = np.asarray(x_padded, dtype=np.float32)
    wt = np.asarray(weight, dtype=np.float32)
    bs = np.asarray(bias, dtype=np.float32)

    xs3 = x[:, -1, :, :]                              # [64, 114, 114]
    win = np.lib.stride_tricks.sliding_window_view(xs3, (KS, KS), axis=(1, 2))
    # [64, 112, 112, 3, 3] -> [64, 9, 12544] with row k = (i, j) shift
    mov_all = win.transpose(0, 3, 4, 1, 2).reshape(B, KS * KS, NPIX)
    # [cores, pairs, img2, 9, seg, SEGW] -> [cores, pairs, seg, (img2, 9), SEGW]
    mov_r = mov_all.reshape(NCORES, PAIRS, 2, KS * KS, NSEG, SEGW)
    mov_h = np.ascontiguousarray(
        mov_r.transpose(0, 1, 4, 2, 3, 5)
    ).reshape(NCORES, PAIRS, NSEG, KDIM, SEGW).astype(ml_dtypes.bfloat16)

    wl = np.ascontiguousarray(wt[:, -1, :, :]).reshape(COUT, KS * KS)
    w2 = np.zeros((128, 128), np.float32)
    for s in range(NSEG):
        w2[32 * s: 32 * s + 9, 0:64] = wl.T
        w2[32 * s + 9: 32 * s + 18, 64:128] = wl.T
    w2 = w2.astype(ml_dtypes.bfloat16)
    b2 = np.tile(bs, 2).reshape(128, 1).astype(np.float32)
    return mov_h, w2, b2


def kernel(x_padded, weight, bias, in_height=112, in_width=112, **_unused):
    from concourse.bass_utils import run_bass_kernel_spmd

    mov_h, w2, b2 = _prep_inputs(x_padded, weight, bias)
    nc = _get_nc()
    in_maps = [
        {"mv": mov_h[c], "w2": w2, "b2": b2}
        for c in range(NCORES)
    ]
    res = run_bass_kernel_spmd(nc, in_maps, core_ids=list(range(NCORES)))
    outs = [
        np.asarray(res.results[c]["out"]).reshape(BL, COUT, H, W)
        for c in range(NCORES)
    ]
    return np.concatenate(outs, axis=0)
